# revision 31
# baseline (speedup 1.0000x reference)
"""Trainium2 Bass kernel for DacVectorQuantize (vq_codebook).

Per-core computation (channel-major layout; 2 batches per core concatenated
along T):
  proj = W1 @ h + b1                       [8, Tc]   (PE fp32, K=1024)
  n/2  = sqrt(0.25 * sum(proj^2, ch))      [1, Tc]   (PE ones-matmul + ACT sqrt)
  dist''[tok,code] = proj.T @ cbn + (n/2)^T @ c      (PE K=8 fp32 + K=1 fp32r)
     -- argmax_code dist'' == reference argmax (monotone per-token transform;
        the c-term only resolves ~1e-7-level ties so fp32r precision suffices)
  idx  = argmax_code dist''                (DVE max8 + max_index)
  quant = codebook[idx]                    (gpsimd indirect gather + PE transpose)
  q_st = proj + (quant - proj)             (DVE)
  out  = W2 @ q_st + b2                    (PE K=8 fp32r + ACT bias-fused copy)
  loss partial = sum((quant - proj)^2)     (ACT Square with accum_out)
Host shards batch over 8 cores, preps packed constants, sums loss partials.

Scheduling constraint: a self-loading fp32 Matmult admits only ONE sync-wait
slot in walrus codegen.  The structure below keeps every matmul at <=1 wait:
all small producers feeding PE are funneled through ACT (one semaphore), and
two warmup matmuls absorb the const-pack DMA waits at start.
"""

import json

import numpy as np

import concourse.bass as bass
import concourse.mybir as mybir
import concourse.tile as tile
from concourse import bass_utils

F32 = mybir.dt.float32
F32R = mybir.dt.float32r
I32 = mybir.dt.int32
U32 = mybir.dt.uint32

B, H, T = 16, 1024, 4096
CD, CS = 8, 1024
NCORES = 8
TC_FULL = B * T // NCORES  # 8192 tokens per core
P = 128
TILE = 512
CHUNK = 128
KT = H // P  # 8 k-tiles for the in-projection
HT = H // P  # 8 h-tiles for the out-projection

# const pack1 (fp32, [128, 1226]) column layout
C_ID = 0          # identity [128, 128]
C_W1T = 128       # in_proj_w^T as [128, k, 8] flattened -> 64 cols
C_B2 = 192        # out_proj_b as [128, 8]
C_B1 = 200        # in_proj_b on rows 0..7
C_ONES = 201      # 1.0 on rows 0..7
C_CBN = 202       # cbn^T [8, 1024] on rows 0..7 AND rows 32..39 (row-packing)
PACK1_W = C_CBN + CS
# const pack2 (fp32r, [40, 2048]): rows 0..7 cols 0..1023 = w2^T;
# rows {0, 32} cols 1024.. = c_j (replicated for row-packed c-term matmuls)
PACK2_H = 40
PACK2_W = 2 * CS


def build_nc(tc_tokens=TC_FULL):
    nc = bass.Bass(trn_type="TRN2")

    ntiles = tc_tokens // TILE
    nchunks_tot = tc_tokens // CHUNK

    h_d = nc.dram_tensor("h", [H, tc_tokens], F32, kind="ExternalInput")
    pack1_d = nc.dram_tensor("pack1", [P, PACK1_W], F32, kind="ExternalInput")
    pack2_d = nc.dram_tensor("pack2", [PACK2_H, PACK2_W], F32R, kind="ExternalInput")
    cb_d = nc.dram_tensor("cb", [CS, CD], F32, kind="ExternalInput")

    out_d = nc.dram_tensor("out", [H, tc_tokens], F32, kind="ExternalOutput")
    proj_d = nc.dram_tensor("proj", [CD, tc_tokens], F32, kind="ExternalOutput")
    idx_d = nc.dram_tensor("indices", [nchunks_tot, CHUNK], I32, kind="ExternalOutput")
    loss_d = nc.dram_tensor("losssum", [CD, 1], F32, kind="ExternalOutput")

    h_t = h_d[:].rearrange("(k p) n -> p k n", p=P)      # [128, 8, Tc]
    out_t = out_d[:].rearrange("(k p) n -> p k n", p=P)  # [128, 8, Tc]

    with tile.TileContext(nc) as tc:
        with (
            tc.tile_pool(name="const", bufs=1) as const,
            tc.tile_pool(name="hpool", bufs=3) as hpool,
            tc.tile_pool(name="encp", bufs=2) as encp,
            tc.tile_pool(name="smalls", bufs=2) as smalls,
            tc.tile_pool(name="mxp", bufs=6) as mxp,
            tc.tile_pool(name="qgp", bufs=10) as qgp,
            tc.tile_pool(name="osb", bufs=3) as osb,
            tc.tile_pool(name="ps_pj", bufs=1, space="PSUM") as ps_pj,
            tc.tile_pool(name="ps_qt", bufs=1, space="PSUM") as ps_qt,
            tc.tile_pool(name="ps_dist", bufs=2, space="PSUM") as ps_dist,
            tc.tile_pool(name="ps_o", bufs=2, space="PSUM") as ps_o,
        ):
            pk1 = const.tile([P, PACK1_W], F32)
            nc.sync.dma_start(pk1[:], pack1_d[:])
            pk2 = const.tile([PACK2_H, PACK2_W], F32R)
            nc.sync.dma_start(pk2[:], pack2_d[:])
            ident = pk1[:, C_ID : C_ID + P]
            b1_ap = pk1[0:CD, C_B1 : C_B1 + 1]
            ones8 = pk1[0:CD, C_ONES : C_ONES + 1]
            idxf = const.tile([P, nchunks_tot], F32)
            losscols = const.tile([CD, ntiles], F32)

            # warmups: absorb the const-pack DMA semaphores so no later
            # matmul needs more than one wait
            wm_ps = ps_pj.tile([8, 8], F32, tag="pj_ps")
            nc.tensor.matmul(wm_ps[:], lhsT=pk1[:, 0:8], rhs=pk1[:, 0:8],
                             start=True, stop=True)
            nc.tensor.matmul(wm_ps[:], lhsT=pk2[0:8, 0:8], rhs=pk2[0:8, 8:16],
                             start=True, stop=True)

            def emit_mm2(qst, t0):
                # out = W2 @ q_st + b2 (fp32r weights, ACT bias-fused copy out)
                for ht in range(HT):
                    o_ps = ps_o.tile([P, TILE], F32, tag="obank")
                    nc.tensor.matmul(
                        o_ps[:],
                        lhsT=pk2[0:CD, ht * P : (ht + 1) * P],
                        rhs=qst[:],
                        start=True,
                        stop=True,
                    )
                    o_sb = osb.tile([P, TILE], F32)
                    nc.scalar.activation(
                        out=o_sb[:],
                        in_=o_ps[:],
                        func=mybir.ActivationFunctionType.Identity,
                        bias=pk1[:, C_B2 + ht : C_B2 + ht + 1],
                        scale=1.0,
                    )
                    nc.sync.dma_start(out_t[:, ht, t0 : t0 + TILE], o_sb[:])

            prev = None
            for ti in range(ntiles):
                t0 = ti * TILE

                h_sb = hpool.tile([P, KT, TILE], F32)
                nc.sync.dma_start(h_sb[:], h_t[:, :, t0 : t0 + TILE])

                # mm1: proj psum [8, TILE]
                pj_ps = ps_pj.tile([CD, TILE], F32, tag="pj_ps")
                for k in range(KT):
                    nc.tensor.matmul(
                        pj_ps[:],
                        lhsT=pk1[:, C_W1T + 8 * k : C_W1T + 8 * k + 8],
                        rhs=h_sb[:, k, :],
                        start=(k == 0),
                        stop=(k == KT - 1),
                    )

                # enc rows = proj + b1 (ACT copy with per-partition bias)
                erep = encp.tile([CD, TILE], F32)
                nc.scalar.activation(
                    out=erep[:],
                    in_=pj_ps[:],
                    func=mybir.ActivationFunctionType.Identity,
                    bias=b1_ap,
                    scale=1.0,
                )
                enc8 = erep[:]
                nc.sync.dma_start(proj_d[:, t0 : t0 + TILE], enc8)

                # channel sum-of-squares -> n/2 (ACT keeps all producers on
                # one semaphore for the PE)
                sq8 = smalls.tile([CD, TILE], F32, tag="sq8")
                nc.scalar.activation(
                    out=sq8[:], in_=enc8,
                    func=mybir.ActivationFunctionType.Square,
                )
                ss_ps = ps_o.tile([1, TILE], F32, tag="obank")
                nc.tensor.matmul(ss_ps[:], lhsT=ones8, rhs=sq8[:],
                                 start=True, stop=True)
                nrep = smalls.tile([1, TILE], F32R, tag="nhalf")
                nc.scalar.activation(
                    out=nrep[:], in_=ss_ps[:],
                    func=mybir.ActivationFunctionType.Sqrt,
                    scale=0.25,
                )

                # similarity matmuls; mm2 of the previous tile is emitted
                # between the two pair groups so the PE stays dense while
                # argmax runs on the DVE
                def sim_pair(cp):
                    a0, b0 = 2 * cp * CHUNK, (2 * cp + 1) * CHUNK
                    dist_a = ps_dist.tile([P, CS], F32, tag="dist_ps")
                    dist_b = ps_dist.tile([P, CS], F32, tag="dist_ps")
                    for dist, c0 in ((dist_a, a0), (dist_b, b0)):
                        for half in range(2):
                            hs = half * 512
                            nc.tensor.matmul(
                                dist[:, hs : hs + 512],
                                lhsT=erep[:, c0 : c0 + CHUNK],
                                rhs=pk1[0:CD, C_CBN + hs : C_CBN + hs + 512],
                                start=True, stop=False,
                            )
                            nc.tensor.matmul(
                                dist[:, hs : hs + 512],
                                lhsT=nrep[:, c0 : c0 + CHUNK],
                                rhs=pk2[0:1, CS + hs : CS + hs + 512],
                                start=False, stop=True,
                            )
                    return [dist_a, dist_b]

                def argmax_gather(c, dist_ps):
                    ci = ti * (TILE // CHUNK) + c
                    mx8 = mxp.tile([P, 8], F32, tag="mx")
                    nc.vector.max(out=mx8[:], in_=dist_ps[:])
                    idx8 = mxp.tile([P, 8], U32, tag="idx")
                    nc.vector.max_index(out=idx8[:], in_max=mx8[:], in_values=dist_ps[:])
                    nc.vector.tensor_copy(out=idxf[:, ci : ci + 1], in_=idx8[:, 0:1])
                    qg = qgp.tile([P, CD], F32)
                    nc.gpsimd.indirect_dma_start(
                        out=qg[:],
                        out_offset=None,
                        in_=cb_d[:],
                        in_offset=bass.IndirectOffsetOnAxis(ap=idx8[:, 0:1], axis=0),
                    )
                    qgs.append(qg)

                def finish_prev(pv):
                    # quant transpose + straight-through + loss + mm2 for the
                    # PREVIOUS tile (its gathers landed long ago, so none of
                    # these PE ops stall; split around sim_pair(1) to keep the
                    # PE dense while this tile's argmax runs on the DVE)
                    pqgs, penc8, pti, pt0 = pv
                    qt_ps = ps_qt.tile([CD, TILE], F32)
                    for c in range(TILE // CHUNK):
                        nc.tensor.transpose(
                            out=qt_ps[:, c * CHUNK : (c + 1) * CHUNK],
                            in_=pqgs[c][:], identity=ident,
                        )
                    d8 = smalls.tile([CD, TILE], F32, tag="d8")
                    nc.vector.tensor_sub(d8[:], qt_ps[:], penc8)
                    qst = smalls.tile([CD, TILE], F32R, tag="qst")
                    nc.vector.tensor_add(qst[:], penc8, d8[:])
                    d2s = smalls.tile([CD, TILE], F32, tag="d2s")
                    nc.scalar.activation(
                        out=d2s[:],
                        in_=d8[:],
                        func=mybir.ActivationFunctionType.Square,
                        accum_out=losscols[:, pti : pti + 1],
                    )
                    return (qst, pt0)

                qgs = []
                d01 = sim_pair(0)
                argmax_gather(0, d01[0])
                argmax_gather(1, d01[1])
                mm2_args = finish_prev(prev) if prev is not None else None
                d23 = sim_pair(1)
                argmax_gather(2, d23[0])
                argmax_gather(3, d23[1])
                if mm2_args is not None:
                    emit_mm2(*mm2_args)
                prev = (qgs, enc8, ti, t0)

            emit_mm2(*finish_prev(prev))

            # ---- finish: losses + indices ----
            loss_sb = const.tile([CD, 1], F32)
            nc.vector.tensor_reduce(
                out=loss_sb[:], in_=losscols[:], axis=mybir.AxisListType.X,
                op=mybir.AluOpType.add,
            )
            nc.sync.dma_start(loss_d[:], loss_sb[:])

            it_ps = ps_pj.tile([nchunks_tot, P], F32, tag="pj_ps")
            nc.tensor.transpose(out=it_ps[:], in_=idxf[:], identity=ident)
            idxi = const.tile([nchunks_tot, P], I32)
            nc.vector.tensor_copy(out=idxi[:], in_=it_ps[:])
            nc.sync.dma_start(idx_d[:], idxi[:])

    nc.finalize()
    _patch_multiwait(nc)
    return nc


def _patch_multiwait(nc):
    """walrus codegen in this toolchain admits only ONE sync-wait per
    instruction.  Split every multi-wait instruction by hoisting the extra
    waits onto NoOp instructions inserted immediately before it on the same
    engine queue (semantically identical: queue order guarantees the waits
    execute before the instruction).  Applied as a BIR-JSON rewrite; the
    patched bytes are pinned onto the instance's to_json_bytes, which both
    the PJRT (axon) and native compile paths call."""
    bir = json.loads(nc.to_json_bytes())
    ctr = 0
    for f in bir["functions"]:
        for b in f["blocks"]:
            insts = b["instructions"]
            out = []
            changed = False
            for inst in insts:
                si = inst.get("sync_info")
                waits = (si or {}).get("on_wait") or []
                if len(waits) > 1:
                    changed = True
                    for w in waits[:-1]:
                        ctr += 1
                        out.append({
                            "name": f"nopw-{ctr}",
                            "opcode": "NoOp",
                            "engine": inst["engine"],
                            "ins": [],
                            "outs": [],
                            "sync_info": {"on_update": [], "on_wait": [w]},
                        })
                    si["on_wait"] = [waits[-1]]
                out.append(inst)
            if changed:
                b["instructions"] = out
    fixed = json.dumps(bir).encode()
    nc.to_json_bytes = lambda: fixed
    return nc


def _host_prep(in_proj_w, in_proj_b, out_proj_w, out_proj_b, codebook):
    cb = np.ascontiguousarray(codebook, dtype=np.float32)
    n64 = np.sqrt((cb.astype(np.float64) ** 2).sum(axis=1, keepdims=True))
    cbn = (cb.astype(np.float64) / np.maximum(n64, 1e-12)).astype(np.float32)
    c = (cbn.astype(np.float64) ** 2).sum(axis=1).astype(np.float32)

    pack1 = np.zeros((P, PACK1_W), dtype=np.float32)
    pack1[:, C_ID : C_ID + P] = np.eye(P, dtype=np.float32)
    pack1[:, C_W1T : C_W1T + H // P * CD] = (
        np.ascontiguousarray(in_proj_w.T, dtype=np.float32).reshape(KT, P, CD)
        .transpose(1, 0, 2).reshape(P, KT * CD)
    )
    pack1[:, C_B2 : C_B2 + HT] = np.asarray(out_proj_b, dtype=np.float32).reshape(HT, P).T
    pack1[0:CD, C_B1] = np.asarray(in_proj_b, dtype=np.float32)
    pack1[0:CD, C_ONES] = 1.0
    pack1[0:CD, C_CBN : C_CBN + CS] = cbn.T
    pack1[32 : 32 + CD, C_CBN : C_CBN + CS] = cbn.T

    pack2 = np.zeros((PACK2_H, PACK2_W), dtype=np.float32)
    pack2[0:CD, 0:H] = np.ascontiguousarray(out_proj_w.T, dtype=np.float32)
    pack2[0, CS : 2 * CS] = c
    pack2[32, CS : 2 * CS] = c
    return {"pack1": pack1, "pack2": pack2, "cb": cb}


_NC_CACHE = {}


def kernel(hidden_state, in_proj_w, in_proj_b, out_proj_w, out_proj_b, codebook,
           _trace=False):
    hidden_state = np.asarray(hidden_state, dtype=np.float32)
    consts = _host_prep(
        np.asarray(in_proj_w), np.asarray(in_proj_b),
        np.asarray(out_proj_w), np.asarray(out_proj_b), np.asarray(codebook),
    )

    key = TC_FULL
    if key not in _NC_CACHE:
        _NC_CACHE[key] = build_nc(TC_FULL)
    nc = _NC_CACHE[key]

    bpc = B // NCORES  # batches per core
    in_maps = []
    for core in range(NCORES):
        hc = np.concatenate(
            [hidden_state[core * bpc + j] for j in range(bpc)], axis=1
        )  # [H, TC_FULL]
        in_maps.append({"h": np.ascontiguousarray(hc), **consts})

    res = bass_utils.run_bass_kernel_spmd(
        nc, in_maps, core_ids=list(range(NCORES)), trace=_trace,
    )
    results = res.results

    out = np.empty((B, H, T), dtype=np.float32)
    proj = np.empty((B, CD, T), dtype=np.float32)
    indices = np.empty((B, T), dtype=np.int32)
    loss_sum = 0.0
    for core in range(NCORES):
        r = results[core]
        for j in range(bpc):
            b = core * bpc + j
            out[b] = r["out"][:, j * T : (j + 1) * T]
            proj[b] = r["proj"][:, j * T : (j + 1) * T]
            indices[b] = r["indices"].reshape(-1)[j * T : (j + 1) * T]
        loss_sum += r["losssum"].astype(np.float64).sum()

    loss = np.float32(loss_sum / (B * CD * T))
    if _trace:
        kernel._last_trace = res
    return (out, loss, loss, indices, proj)


# revision 37
# speedup vs baseline: 1.4008x; 1.4008x over previous
"""Trainium2 Bass kernel for DacVectorQuantize (vq_codebook).

Per-core computation (channel-major layout; 2 batches per core concatenated
along T):
  proj = W1 @ h + b1                       [8, Tc]   (PE fp32, K=1024)
  n/2  = sqrt(0.25 * sum(proj^2, ch))      [1, Tc]   (PE ones-matmul + ACT sqrt)
  dist''[tok,code] = proj.T @ cbn + (n/2)^T @ c      (PE K=8 fp32 + K=1 fp32r)
     -- argmax_code dist'' == reference argmax (monotone per-token transform;
        the c-term only resolves ~1e-7-level ties so fp32r precision suffices)
  idx  = argmax_code dist''                (DVE max8 + max_index)
  quant = codebook[idx]                    (gpsimd indirect gather + PE transpose)
  q_st = proj + (quant - proj)             (DVE)
  out  = W2 @ q_st + b2                    (PE K=8 fp32r + ACT bias-fused copy)
  loss partial = sum((quant - proj)^2)     (ACT Square with accum_out)
Host shards batch over 8 cores, preps packed constants, sums loss partials.

Scheduling constraint: a self-loading fp32 Matmult admits only ONE sync-wait
slot in walrus codegen.  The structure below keeps every matmul at <=1 wait:
all small producers feeding PE are funneled through ACT (one semaphore), and
two warmup matmuls absorb the const-pack DMA waits at start.
"""

import json

import numpy as np

import concourse.bass as bass
import concourse.mybir as mybir
import concourse.tile as tile
from concourse import bass_utils

F32 = mybir.dt.float32
F32R = mybir.dt.float32r
F16 = mybir.dt.float16
I32 = mybir.dt.int32
U32 = mybir.dt.uint32

B, H, T = 16, 1024, 4096
CD, CS = 8, 1024
NCORES = 8
TC_FULL = B * T // NCORES  # 8192 tokens per core
P = 128
TILE = 512
CHUNK = 128
KT = H // P  # 8 k-tiles for the in-projection
HT = H // P  # 8 h-tiles for the out-projection

# const pack1 (fp32, [128, 1226]) column layout
C_ID = 0          # identity [128, 128]
C_W1T = 128       # in_proj_w^T as [128, k, 8] flattened -> 64 cols
C_B2 = 192        # out_proj_b as [128, 8]
C_B1 = 200        # in_proj_b on rows 0..7
C_ONES = 201      # 1.0 on rows 0..7
C_CBN = 202       # cbn^T [8, 1024] on rows 0..7 AND rows 32..39 (row-packing)
PACK1_W = C_CBN + CS
# const pack2 (fp32r, [40, 2048]): rows 0..7 cols 0..1023 = w2^T;
# rows {0, 32} cols 1024.. = c_j (replicated for row-packed c-term matmuls)
PACK2_H = 40
PACK2_W = 2 * CS


def build_nc(tc_tokens=TC_FULL):
    nc = bass.Bass(trn_type="TRN2")

    ntiles = tc_tokens // TILE
    nchunks_tot = tc_tokens // CHUNK

    h_d = nc.dram_tensor("h", [H, tc_tokens], F32, kind="ExternalInput")
    pack1_d = nc.dram_tensor("pack1", [P, PACK1_W], F32, kind="ExternalInput")
    pack2_d = nc.dram_tensor("pack2", [PACK2_H, PACK2_W], F32R, kind="ExternalInput")
    # fp16 hi/lo split of cbn^T: cols 0..CS-1 = hi, CS..2CS-1 = lo
    pack3_d = nc.dram_tensor("pack3", [CD, 2 * CS], F16, kind="ExternalInput")
    cb_d = nc.dram_tensor("cb", [CS, CD], F32, kind="ExternalInput")

    out_d = nc.dram_tensor("out", [H, tc_tokens], F32, kind="ExternalOutput")
    proj_d = nc.dram_tensor("proj", [CD, tc_tokens], F32, kind="ExternalOutput")
    idx_d = nc.dram_tensor("indices", [nchunks_tot, CHUNK], I32, kind="ExternalOutput")
    loss_d = nc.dram_tensor("losssum", [CD, 1], F32, kind="ExternalOutput")

    h_t = h_d[:].rearrange("(k p) n -> p k n", p=P)      # [128, 8, Tc]
    out_t = out_d[:].rearrange("(k p) n -> p k n", p=P)  # [128, 8, Tc]

    with tile.TileContext(nc) as tc:
        with (
            tc.tile_pool(name="const", bufs=1) as const,
            tc.tile_pool(name="hpool", bufs=3) as hpool,
            tc.tile_pool(name="encp", bufs=2) as encp,
            tc.tile_pool(name="smalls", bufs=2) as smalls,
            tc.tile_pool(name="mxp", bufs=6) as mxp,
            tc.tile_pool(name="qgp", bufs=10) as qgp,
            tc.tile_pool(name="osb", bufs=3) as osb,
            tc.tile_pool(name="ps_pj", bufs=1, space="PSUM") as ps_pj,
            tc.tile_pool(name="ps_qt", bufs=1, space="PSUM") as ps_qt,
            tc.tile_pool(name="ps_dist", bufs=2, space="PSUM") as ps_dist,
            tc.tile_pool(name="ps_o", bufs=2, space="PSUM") as ps_o,
        ):
            pk1 = const.tile([P, PACK1_W], F32)
            nc.sync.dma_start(pk1[:], pack1_d[:])
            pk2 = const.tile([PACK2_H, PACK2_W], F32R)
            nc.sync.dma_start(pk2[:], pack2_d[:])
            pk3 = const.tile([CD, 2 * CS], F16)
            nc.sync.dma_start(pk3[:], pack3_d[:])
            ident = pk1[:, C_ID : C_ID + P]
            b1_ap = pk1[0:CD, C_B1 : C_B1 + 1]
            ones8 = pk1[0:CD, C_ONES : C_ONES + 1]
            idxf = const.tile([P, nchunks_tot], F32)
            losscols = const.tile([CD, ntiles], F32)

            # warmups: absorb the const-pack DMA semaphores so no later
            # matmul needs more than one wait
            wm_ps = ps_pj.tile([8, 8], F32, tag="pj_ps")
            nc.tensor.matmul(wm_ps[:], lhsT=pk1[:, 0:8], rhs=pk1[:, 0:8],
                             start=True, stop=True)
            nc.tensor.matmul(wm_ps[:], lhsT=pk2[0:8, 0:8], rhs=pk2[0:8, 8:16],
                             start=True, stop=True)
            nc.tensor.matmul(wm_ps[:], lhsT=pk3[0:8, 0:8], rhs=pk3[0:8, 8:16],
                             start=True, stop=True)

            def emit_mm2(qst, t0):
                # out = W2 @ q_st + b2 (fp32r weights, ACT bias-fused copy out)
                for ht in range(HT):
                    o_ps = ps_o.tile([P, TILE], F32, tag="obank")
                    nc.tensor.matmul(
                        o_ps[:],
                        lhsT=pk2[0:CD, ht * P : (ht + 1) * P],
                        rhs=qst[:],
                        start=True,
                        stop=True,
                    )
                    o_sb = osb.tile([P, TILE], F32)
                    nc.scalar.activation(
                        out=o_sb[:],
                        in_=o_ps[:],
                        func=mybir.ActivationFunctionType.Identity,
                        bias=pk1[:, C_B2 + ht : C_B2 + ht + 1],
                        scale=1.0,
                    )
                    nc.sync.dma_start(out_t[:, ht, t0 : t0 + TILE], o_sb[:])

            prev = None
            for ti in range(ntiles):
                t0 = ti * TILE

                h_sb = hpool.tile([P, KT, TILE], F32)
                nc.sync.dma_start(h_sb[:], h_t[:, :, t0 : t0 + TILE])

                # mm1: proj psum [8, TILE]
                pj_ps = ps_pj.tile([CD, TILE], F32, tag="pj_ps")
                for k in range(KT):
                    nc.tensor.matmul(
                        pj_ps[:],
                        lhsT=pk1[:, C_W1T + 8 * k : C_W1T + 8 * k + 8],
                        rhs=h_sb[:, k, :],
                        start=(k == 0),
                        stop=(k == KT - 1),
                    )

                # enc rows = proj + b1 (ACT copy with per-partition bias)
                erep = encp.tile([CD, TILE], F32)
                nc.scalar.activation(
                    out=erep[:],
                    in_=pj_ps[:],
                    func=mybir.ActivationFunctionType.Identity,
                    bias=b1_ap,
                    scale=1.0,
                )
                enc8 = erep[:]
                nc.sync.dma_start(proj_d[:, t0 : t0 + TILE], enc8)

                # channel sum-of-squares -> n/2 (ACT keeps all producers on
                # one semaphore for the PE)
                sq8 = smalls.tile([CD, TILE], F32, tag="sq8")
                nc.scalar.activation(
                    out=sq8[:], in_=enc8,
                    func=mybir.ActivationFunctionType.Square,
                )
                ss_ps = ps_o.tile([1, TILE], F32, tag="obank")
                nc.tensor.matmul(ss_ps[:], lhsT=ones8, rhs=sq8[:],
                                 start=True, stop=True)
                nrep = smalls.tile([1, TILE], F32R, tag="nhalf")
                nc.scalar.activation(
                    out=nrep[:], in_=ss_ps[:],
                    func=mybir.ActivationFunctionType.Sqrt,
                    scale=0.25,
                )

                # fp16 hi/lo split of enc: enc = e1 + e2 with e1 = f16(enc),
                # e2 = f16(enc - e1).  enc.cbn is then computed as
                # e1.c1 + e1.c2 + e2.c1 (each product exact in the PE's
                # internal e10m23 accumulate) -- 3 passes at 1 cyc/row
                # instead of fp32's 4 cyc/row, with ~2^-22 representation
                # error (at the fp32 rounding-noise level).
                e1f = smalls.tile([CD, TILE], F16, tag="e1f")
                nc.scalar.activation(
                    out=e1f[:], in_=enc8,
                    func=mybir.ActivationFunctionType.Copy,
                )
                e2f = smalls.tile([CD, TILE], F16, tag="e2f")
                nc.vector.tensor_sub(e2f[:], enc8, e1f[:])

                # similarity matmuls; mm2 of the previous tile is emitted
                # between the two pair groups so the PE stays dense while
                # argmax runs on the DVE
                def sim_pair(cp):
                    a0, b0 = 2 * cp * CHUNK, (2 * cp + 1) * CHUNK
                    dist_a = ps_dist.tile([P, CS], F32, tag="dist_ps")
                    dist_b = ps_dist.tile([P, CS], F32, tag="dist_ps")
                    for dist, c0 in ((dist_a, a0), (dist_b, b0)):
                        for half in range(2):
                            hs = half * 512
                            nc.tensor.matmul(
                                dist[:, hs : hs + 512],
                                lhsT=e1f[:, c0 : c0 + CHUNK],
                                rhs=pk3[:, hs : hs + 512],
                                start=True, stop=False,
                            )
                            nc.tensor.matmul(
                                dist[:, hs : hs + 512],
                                lhsT=e1f[:, c0 : c0 + CHUNK],
                                rhs=pk3[:, CS + hs : CS + hs + 512],
                                start=False, stop=False,
                            )
                            nc.tensor.matmul(
                                dist[:, hs : hs + 512],
                                lhsT=e2f[:, c0 : c0 + CHUNK],
                                rhs=pk3[:, hs : hs + 512],
                                start=False, stop=False,
                            )
                            nc.tensor.matmul(
                                dist[:, hs : hs + 512],
                                lhsT=nrep[:, c0 : c0 + CHUNK],
                                rhs=pk2[0:1, CS + hs : CS + hs + 512],
                                start=False, stop=True,
                            )
                    return [dist_a, dist_b]

                def argmax_gather(c, dist_ps):
                    ci = ti * (TILE // CHUNK) + c
                    mx8 = mxp.tile([P, 8], F32, tag="mx")
                    nc.vector.max(out=mx8[:], in_=dist_ps[:])
                    idx8 = mxp.tile([P, 8], U32, tag="idx")
                    nc.vector.max_index(out=idx8[:], in_max=mx8[:], in_values=dist_ps[:])
                    nc.vector.tensor_copy(out=idxf[:, ci : ci + 1], in_=idx8[:, 0:1])
                    qg = qgp.tile([P, CD], F32)
                    nc.gpsimd.indirect_dma_start(
                        out=qg[:],
                        out_offset=None,
                        in_=cb_d[:],
                        in_offset=bass.IndirectOffsetOnAxis(ap=idx8[:, 0:1], axis=0),
                    )
                    qgs.append(qg)

                def finish_prev(pv):
                    # quant transpose + straight-through + loss + mm2 for the
                    # PREVIOUS tile (its gathers landed long ago, so none of
                    # these PE ops stall; split around sim_pair(1) to keep the
                    # PE dense while this tile's argmax runs on the DVE)
                    pqgs, penc8, pti, pt0 = pv
                    qt_ps = ps_qt.tile([CD, TILE], F32)
                    for c in range(TILE // CHUNK):
                        nc.tensor.transpose(
                            out=qt_ps[:, c * CHUNK : (c + 1) * CHUNK],
                            in_=pqgs[c][:], identity=ident,
                        )
                    d8 = smalls.tile([CD, TILE], F32, tag="d8")
                    nc.vector.tensor_sub(d8[:], qt_ps[:], penc8)
                    qst = smalls.tile([CD, TILE], F32R, tag="qst")
                    nc.vector.tensor_add(qst[:], penc8, d8[:])
                    d2s = smalls.tile([CD, TILE], F32, tag="d2s")
                    nc.scalar.activation(
                        out=d2s[:],
                        in_=d8[:],
                        func=mybir.ActivationFunctionType.Square,
                        accum_out=losscols[:, pti : pti + 1],
                    )
                    return (qst, pt0)

                qgs = []
                d01 = sim_pair(0)
                argmax_gather(0, d01[0])
                argmax_gather(1, d01[1])
                mm2_args = finish_prev(prev) if prev is not None else None
                d23 = sim_pair(1)
                argmax_gather(2, d23[0])
                argmax_gather(3, d23[1])
                if mm2_args is not None:
                    emit_mm2(*mm2_args)
                prev = (qgs, enc8, ti, t0)

            emit_mm2(*finish_prev(prev))

            # ---- finish: losses + indices ----
            loss_sb = const.tile([CD, 1], F32)
            nc.vector.tensor_reduce(
                out=loss_sb[:], in_=losscols[:], axis=mybir.AxisListType.X,
                op=mybir.AluOpType.add,
            )
            nc.sync.dma_start(loss_d[:], loss_sb[:])

            it_ps = ps_pj.tile([nchunks_tot, P], F32, tag="pj_ps")
            nc.tensor.transpose(out=it_ps[:], in_=idxf[:], identity=ident)
            idxi = const.tile([nchunks_tot, P], I32)
            nc.vector.tensor_copy(out=idxi[:], in_=it_ps[:])
            nc.sync.dma_start(idx_d[:], idxi[:])

    nc.finalize()
    _patch_multiwait(nc)
    return nc


def _patch_multiwait(nc):
    """walrus codegen in this toolchain admits only ONE sync-wait per
    instruction.  Split every multi-wait instruction by hoisting the extra
    waits onto NoOp instructions inserted immediately before it on the same
    engine queue (semantically identical: queue order guarantees the waits
    execute before the instruction).  Applied as a BIR-JSON rewrite; the
    patched bytes are pinned onto the instance's to_json_bytes, which both
    the PJRT (axon) and native compile paths call."""
    bir = json.loads(nc.to_json_bytes())
    ctr = 0
    for f in bir["functions"]:
        for b in f["blocks"]:
            insts = b["instructions"]
            out = []
            changed = False
            for inst in insts:
                si = inst.get("sync_info")
                waits = (si or {}).get("on_wait") or []
                if len(waits) > 1:
                    changed = True
                    for w in waits[:-1]:
                        ctr += 1
                        out.append({
                            "name": f"nopw-{ctr}",
                            "opcode": "NoOp",
                            "engine": inst["engine"],
                            "ins": [],
                            "outs": [],
                            "sync_info": {"on_update": [], "on_wait": [w]},
                        })
                    si["on_wait"] = [waits[-1]]
                out.append(inst)
            if changed:
                b["instructions"] = out
    fixed = json.dumps(bir).encode()
    nc.to_json_bytes = lambda: fixed
    return nc


def _host_prep(in_proj_w, in_proj_b, out_proj_w, out_proj_b, codebook):
    cb = np.ascontiguousarray(codebook, dtype=np.float32)
    n64 = np.sqrt((cb.astype(np.float64) ** 2).sum(axis=1, keepdims=True))
    cbn = (cb.astype(np.float64) / np.maximum(n64, 1e-12)).astype(np.float32)
    c = (cbn.astype(np.float64) ** 2).sum(axis=1).astype(np.float32)

    pack1 = np.zeros((P, PACK1_W), dtype=np.float32)
    pack1[:, C_ID : C_ID + P] = np.eye(P, dtype=np.float32)
    pack1[:, C_W1T : C_W1T + H // P * CD] = (
        np.ascontiguousarray(in_proj_w.T, dtype=np.float32).reshape(KT, P, CD)
        .transpose(1, 0, 2).reshape(P, KT * CD)
    )
    pack1[:, C_B2 : C_B2 + HT] = np.asarray(out_proj_b, dtype=np.float32).reshape(HT, P).T
    pack1[0:CD, C_B1] = np.asarray(in_proj_b, dtype=np.float32)
    pack1[0:CD, C_ONES] = 1.0
    pack1[0:CD, C_CBN : C_CBN + CS] = cbn.T
    pack1[32 : 32 + CD, C_CBN : C_CBN + CS] = cbn.T

    pack2 = np.zeros((PACK2_H, PACK2_W), dtype=np.float32)
    pack2[0:CD, 0:H] = np.ascontiguousarray(out_proj_w.T, dtype=np.float32)
    pack2[0, CS : 2 * CS] = c
    pack2[32, CS : 2 * CS] = c

    cbn_hi = cbn.astype(np.float16)
    cbn_lo = (cbn - cbn_hi.astype(np.float32)).astype(np.float16)
    pack3 = np.zeros((CD, 2 * CS), dtype=np.float16)
    pack3[:, 0:CS] = cbn_hi.T
    pack3[:, CS : 2 * CS] = cbn_lo.T
    return {"pack1": pack1, "pack2": pack2, "pack3": pack3, "cb": cb}


_NC_CACHE = {}


def kernel(hidden_state, in_proj_w, in_proj_b, out_proj_w, out_proj_b, codebook,
           _trace=False):
    hidden_state = np.asarray(hidden_state, dtype=np.float32)
    consts = _host_prep(
        np.asarray(in_proj_w), np.asarray(in_proj_b),
        np.asarray(out_proj_w), np.asarray(out_proj_b), np.asarray(codebook),
    )

    key = TC_FULL
    if key not in _NC_CACHE:
        _NC_CACHE[key] = build_nc(TC_FULL)
    nc = _NC_CACHE[key]

    bpc = B // NCORES  # batches per core
    in_maps = []
    for core in range(NCORES):
        hc = np.concatenate(
            [hidden_state[core * bpc + j] for j in range(bpc)], axis=1
        )  # [H, TC_FULL]
        in_maps.append({"h": np.ascontiguousarray(hc), **consts})

    res = bass_utils.run_bass_kernel_spmd(
        nc, in_maps, core_ids=list(range(NCORES)), trace=_trace,
    )
    results = res.results

    out = np.empty((B, H, T), dtype=np.float32)
    proj = np.empty((B, CD, T), dtype=np.float32)
    indices = np.empty((B, T), dtype=np.int32)
    loss_sum = 0.0
    for core in range(NCORES):
        r = results[core]
        for j in range(bpc):
            b = core * bpc + j
            out[b] = r["out"][:, j * T : (j + 1) * T]
            proj[b] = r["proj"][:, j * T : (j + 1) * T]
            indices[b] = r["indices"].reshape(-1)[j * T : (j + 1) * T]
        loss_sum += r["losssum"].astype(np.float64).sum()

    loss = np.float32(loss_sum / (B * CD * T))
    if _trace:
        kernel._last_trace = res
    return (out, loss, loss, indices, proj)
